# revision 1
# baseline (speedup 1.0000x reference)
"""Trainium2 Bass kernel for nn_Block1 (dense_cnn edge-filter bank).

kernel(pan) -> [2, 6, 2048, 2048] f32: concat([diff_y, diff_x, roberts,
prewitt, sobel, laplacian], axis=1) with a global-max normalization of the
Gaussian-filtered image (see the reference nn.Module).

Distribution: pure data parallel over 8 NeuronCores — core c owns rows
[512*(c%4), 512*(c%4+1)) of batch item c//4. Each core gets a reflect-padded
[516, 2050] f32 input slab; the per-batch global max is combined with an
on-device AllReduce(max) collective. Filter weights are baked in as banded
[128,128] matrices (conv-as-matmul along rows; horizontal taps via shifted
rhs reads into accumulating matmuls).

Per-core pipeline (5 row-tiles of 128):
  phase A: gauss via 3 accumulating f32r banded MMs -> LP; running max via
           DVE reduce; dy via f32r banded MM (stored); dx via GPSIMD sub.
  norm:    GPSIMD partition_all_reduce + AllReduce(max) over 8 cores;
           r = 255/norm and scale = norm/255 broadcast per partition.
  phase B: spf = floor(r*LP) via RNE-cast + compare fixup (custom DVE op);
           sobel on LP (f32r bands; r applied inside ACT Abs scale);
           prewitt/roberts/laplace on spf (bf16 bands — exact integer math);
           edge_response = ACT Abs -> u8 (saturating RNE = round+clip);
           add_weighted  = custom DVE (a+b)*0.5 -> u8 (RNE writeback);
           final scale via tensor_scalar; stores on HWDGE/SWDGE.
"""
import sys

sys.path.insert(0, "/opt/trn_rl_repo")

import numpy as np
import ml_dtypes

import concourse.bacc as bacc
import concourse.mybir as mybir
import concourse.bass_isa as bass_isa
from concourse.tile import TileContext
from concourse.dve_ops import (DveOp, DveOpSpec, OPS, CUSTOM_DVE_SPECS,
                               _SUB_OPCODE_FOR_NAME, _CUSTOM_DVE_ROW_BASE)
from concourse.dve_spec import Spec, Src0, Src1, C0, C1, C2, One, select, lower

f32 = mybir.dt.float32
f32r = mybir.dt.float32r
bf16 = mybir.dt.bfloat16
i32 = mybir.dt.int32
u8 = mybir.dt.uint8

P = 128
W = 2048
WP = 2050
ROWS = 516
NT = 5
NCHUNK = 4
TSTART = [0, 124, 248, 372, 388]
STORE = [(0, 124, 0, 124), (0, 124, 124, 248), (0, 124, 248, 372),
         (0, 124, 372, 496), (108, 124, 496, 512)]
NCORES = 8


# --------------- custom DVE ops (registered once per process) -------------- #

def _register(name, spec):
    if name in _SUB_OPCODE_FOR_NAME:
        for op in OPS:
            if op.name == name:
                return op
    shas = {}
    for ver in ("v3", "v4"):
        s = DveOpSpec(name=name, opcode=0, uops=lower(spec, ver=ver), rd1_en=False)
        shas[ver] = s.sha(ver)
    op = DveOp(name, spec, subdim=False, uops_sha=shas)
    OPS.append(op)
    CUSTOM_DVE_SPECS[name] = spec
    _SUB_OPCODE_FOR_NAME[name] = _CUSTOM_DVE_ROW_BASE + len(OPS) - 1
    return op


# (in0 + in1) * imm2; rounding/clipping via u8 writeback (saturating RNE)
ADD_SCALE_ANT = _register("ADD_SCALE_ANT", Spec(
    body=(Src0 + Src1) * C2,
    reference=lambda in0, in1, s0, s1, imm2: (in0 + in1) * imm2,
))

# floor fixup: in0 = rint(in1*s0) (prior cast pass); out = floor(in1*s0)
FLOOR_FIX_ANT = _register("FLOOR_FIX_ANT", Spec(
    body=select(Src0 > Src1 * C0, Src0 - One, Src0),
    reference=lambda in0, in1, s0, s1, imm2: in0 - (in0 > in1 * s0),
))


# ------------------------------- bass program ------------------------------ #

def _band(c):
    """Shifted banded matrix: A[k, m] = c[k-m] for k-m in {0,1,2}:
    out[m] = sum_t c[t] * x[m+t] (3-tap vertical conv centered at row m+1)."""
    A = np.zeros((P, P), np.float32)
    for m in range(P):
        for t in range(3):
            if m + t < P:
                A[m + t, m] = c[t]
    return A


def _emit_filter(nc, ps, spec, rhs, h=0):
    """Accumulating banded MMs for chunks [2h, 2h+1] into a [P, 1024] psum
    half, grouped by band (LDW reuse). spec: [(band_tile, dx)] taps."""
    writes = [0, 0]
    total = len(spec)
    for bd, dx in spec:
        for i, c in enumerate((2 * h, 2 * h + 1)):
            first = writes[i] == 0
            writes[i] += 1
            last = writes[i] == total
            nc.tensor.matmul(ps[:, 512 * i:512 * (i + 1)], bd[:],
                             rhs[:, 512 * c + dx:512 * c + dx + 512],
                             start=first, stop=last)


def _build():
    nc = bacc.Bacc("TRN2", num_devices=NCORES)
    X = nc.dram_tensor("x", [ROWS, WP], f32, kind="ExternalInput")
    BSEL = nc.dram_tensor("bsel", [1, 2], f32, kind="ExternalInput")
    O = nc.dram_tensor("o", [2, 512, W], f32, kind="ExternalOutput")
    O8 = nc.dram_tensor("o8", [4, 512, W], u8, kind="ExternalOutput")
    NORMS = nc.dram_tensor("onorms", [1, 2], f32, kind="ExternalOutput")

    G1m = nc.inline_tensor(_band([1, 2, 1]) / 16.0, name="G1m")
    G2m = nc.inline_tensor(_band([2, 4, 2]) / 16.0, name="G2m")
    SB1m = nc.inline_tensor(_band([-1, 0, 1]), name="SB1m")
    SB2m = nc.inline_tensor(_band([-2, 0, 2]), name="SB2m")
    BPm = nc.inline_tensor(_band([1, 2, 1]), name="BPm")
    BNm = nc.inline_tensor(_band([-1, -2, -1]), name="BNm")
    DYm = nc.inline_tensor(_band([0, -1, 1]), name="DYm")
    bfb = lambda c, nm: nc.inline_tensor(_band(c).astype(ml_dtypes.bfloat16),
                                         name=nm)
    DPb = bfb([1, 0, -1], "DPb")
    BXPb = bfb([1, 1, 1], "BXPb")
    BXNb = bfb([-1, -1, -1], "BXNb")
    Ib = bfb([0, 1, 0], "Ib")
    SHNb = bfb([-1, 0, 0], "SHNb")
    L2b = bfb([2, 0, 2], "L2b")
    M8b = bfb([0, -8, 0], "M8b")

    with TileContext(nc) as tc:
        with tc.tile_pool(name="keep", bufs=1) as keep, \
             tc.tile_pool(name="work", bufs=2) as work, \
             tc.tile_pool(name="out", bufs=4) as outp, \
             tc.tile_pool(name="u8", bufs=4) as u8p, \
             tc.tile_pool(name="psum", bufs=4, space="PSUM") as psum, \
             tc.tile_pool(name="dram", bufs=1, space="DRAM") as dram:

            def load_const(t, dt_):
                tl = keep.tile([P, P], dt_, tag=t.name)
                nc.sync.dma_start(out=tl[:], in_=t.ap()[:, :])
                return tl

            # X loads first (phase-A critical path), split across both HWDGE
            # engines so they run in parallel
            masters = {}
            for m in (G1m, G2m):
                masters[m.name] = load_const(m, f32)
            xts = []
            for t in range(NT):
                xt = keep.tile([P, WP], f32, tag=f"x{t}")
                eng = nc.sync if t % 2 == 0 else nc.scalar
                eng.dma_start(out=xt[:], in_=X[TSTART[t]:TSTART[t] + P, :])
                xts.append(xt)
            rbands = {}
            for nm in ("G1m", "G2m"):
                rt = keep.tile([P, P], f32r, tag=nm + "r")
                nc.vector.tensor_copy(out=rt[:], in_=masters[nm][:])
                rbands[nm] = rt
            xr0 = work.tile([P, WP], f32r, tag="xr")
            nc.vector.tensor_copy(out=xr0[:], in_=xts[0][:])

            for m in (SB1m, SB2m, BPm, BNm, DYm):
                masters[m.name] = load_const(m, f32)
            DPh = load_const(DPb, bf16)
            BXPh = load_const(BXPb, bf16)
            BXNh = load_const(BXNb, bf16)
            Ih = load_const(Ib, bf16)
            SHNh = load_const(SHNb, bf16)
            L2h = load_const(L2b, bf16)
            M8h = load_const(M8b, bf16)
            bsel = keep.tile([1, 2], f32, tag="bsel")
            nc.sync.dma_start(out=bsel[:], in_=BSEL[:, :])
            bselb = keep.tile([P, 2], f32, tag="bselb")
            nc.gpsimd.partition_broadcast(bselb[:], bsel[:], P)

            for nm in ("SB1m", "SB2m", "BPm", "BNm", "DYm"):
                rt = keep.tile([P, P], f32r, tag=nm + "r")
                nc.vector.tensor_copy(out=rt[:], in_=masters[nm][:])
                rbands[nm] = rt
            G1r, G2r = rbands["G1m"], rbands["G2m"]
            SB1r, SB2r = rbands["SB1m"], rbands["SB2m"]
            BPr, BNr, DYr = rbands["BPm"], rbands["BNm"], rbands["DYm"]

            lps = []
            macc = keep.tile([P, 1], f32, tag="macc")

            # ============ phase A: gauss + running max + dy ============ #
            for t in range(NT):
                xt = xts[t]
                if t == 0:
                    xr = xr0
                else:
                    xr = work.tile([P, WP], f32r, tag="xr")
                    nc.vector.tensor_copy(out=xr[:], in_=xt[:])
                lp = keep.tile([P, WP], f32r, tag=f"lp{t}")
                H = W // 2
                for h in (0, 1):
                    ps = psum.tile([P, H], f32, tag="ps")
                    _emit_filter(nc, ps[:], [(G1r, 0), (G1r, 2), (G2r, 1)], xr, h)
                    nc.scalar.activation(lp[:, 1 + H * h:1 + H * (h + 1)], ps[:],
                                         mybir.ActivationFunctionType.Copy)
                    mt = work.tile([P, 1], f32, tag="mt")
                    nc.vector.tensor_reduce(out=mt[:], in_=ps[:],
                                            axis=mybir.AxisListType.X,
                                            op=mybir.AluOpType.max)
                    if t == 0 and h == 0:
                        nc.vector.tensor_copy(out=macc[:], in_=mt[:])
                    else:
                        nc.vector.tensor_tensor(out=macc[:], in0=macc[:],
                                                in1=mt[:],
                                                op=mybir.AluOpType.max)
                nc.vector.tensor_copy(out=lp[:, 0:1], in_=lp[:, 2:3])
                nc.vector.tensor_copy(out=lp[:, W + 1:W + 2], in_=lp[:, W - 1:W])
                lps.append(lp)
                j0, j1, g0, g1 = STORE[t]
                dyf = outp.tile([P, W], f32, tag="of")
                for h in (0, 1):
                    pdy = psum.tile([P, H], f32, tag="ps")
                    _emit_filter(nc, pdy[:], [(DYr, 1)], xr, h)
                    nc.scalar.activation(dyf[:, H * h:H * (h + 1)], pdy[:],
                                         mybir.ActivationFunctionType.Copy)
                (nc.sync if j0 == 0 else nc.gpsimd).dma_start(
                    out=O[0, g0:g1, :], in_=dyf[j0:j1])

            # pre-issue sobel MM groups for tile 0 (norm-independent) so PE
            # has work around the collective stall
            pre_psx, pre_psy = [], []
            for h in (0, 1):
                px = psum.tile([P, W // 2], f32, tag="ps")
                _emit_filter(nc, px[:], [(SB1r, 0), (SB1r, 2), (SB2r, 1)],
                             lps[0], h)
                pre_psx.append(px)
            for h in (0, 1):
                py = psum.tile([P, W // 2], f32, tag="ps")
                _emit_filter(nc, py[:], [(BNr, 0), (BPr, 2)], lps[0], h)
                pre_psy.append(py)

            # ---- norm across partitions and cores ---- #
            # carry [128,2] through the collective so the post-collective
            # chain is pure back-to-back DVE (no partition_broadcast hop)
            pm = keep.tile([P, 1], f32, tag="pm")
            nc.gpsimd.partition_all_reduce(pm[:], macc[:], P,
                                           bass_isa.ReduceOp.max)
            m2 = keep.tile([P, 2], f32, tag="m2")
            nc.vector.tensor_scalar(out=m2[:], in0=bselb[:], scalar1=pm[:, 0:1],
                                    scalar2=None, op0=mybir.AluOpType.mult)
            ib = dram.tile([P, 2], f32)
            ob = dram.tile([P, 2], f32)
            nc.gpsimd.dma_start(ib[:], m2[:])
            nc.gpsimd.collective_compute(
                "AllReduce", mybir.AluOpType.max,
                replica_groups=[list(range(NCORES))],
                ins=[ib.opt()], outs=[ob.opt()])
            norms_pp = keep.tile([P, 2], f32, tag="norms_pp")
            nc.gpsimd.dma_start(norms_pp[:], ob[:])
            nc.gpsimd.dma_start(NORMS[:, :], ob[0:1, :])
            nbv = keep.tile([P, 2], f32, tag="nbv")
            nc.vector.tensor_tensor(out=nbv[:], in0=norms_pp[:], in1=bselb[:],
                                    op=mybir.AluOpType.mult)
            nb = keep.tile([P, 1], f32, tag="nb")
            nc.vector.tensor_reduce(out=nb[:], in_=nbv[:],
                                    axis=mybir.AxisListType.X,
                                    op=mybir.AluOpType.add)
            rcp = keep.tile([P, 1], f32, tag="rcp")
            nc.vector.reciprocal(out=rcp[:], in_=nb[:])
            rb = keep.tile([P, 1], f32, tag="rb")
            nc.vector.tensor_scalar(out=rb[:], in0=rcp[:], scalar1=255.0,
                                    scalar2=None, op0=mybir.AluOpType.mult)

            # ---- dx: norm-independent, overlaps the collective ---- #
            for t in range(NT):
                j0, j1, g0, g1 = STORE[t]
                xt = xts[t]
                dxf = outp.tile([P, W], f32, tag="of")
                nc.gpsimd.tensor_tensor(out=dxf[:], in0=xt[:, 1:W + 1],
                                        in1=xt[:, 0:W],
                                        op=mybir.AluOpType.subtract)
                nc.gpsimd.dma_start(out=O[1, g0:g1, :], in_=dxf[j0 + 2:j1 + 2])

            # ========================= phase B ========================= #
            def edge_pair(spec_x, spec_y, rhs, ch, t, abs_scale, scale_eng=None,
                          pre=None):
                j0, j1, g0, g1 = STORE[t]
                H = W // 2
                ax = u8p.tile([P, W], u8, tag="ax")
                ay = u8p.tile([P, W], u8, tag="ay")
                for h in (0, 1):
                    if pre is not None:
                        psx = pre[0][h]
                    else:
                        psx = psum.tile([P, H], f32, tag="ps")
                        _emit_filter(nc, psx[:], spec_x, rhs, h)
                    nc.scalar.activation(ax[:, H * h:H * (h + 1)], psx[:],
                                         mybir.ActivationFunctionType.Abs,
                                         scale=abs_scale)
                for h in (0, 1):
                    if pre is not None:
                        psy = pre[1][h]
                    else:
                        psy = psum.tile([P, H], f32, tag="ps")
                        _emit_filter(nc, psy[:], spec_y, rhs, h)
                    nc.scalar.activation(ay[:, H * h:H * (h + 1)], psy[:],
                                         mybir.ActivationFunctionType.Abs,
                                         scale=abs_scale)
                s8 = u8p.tile([P, W], u8, tag="s8")
                nc.vector._custom_dve(ADD_SCALE_ANT, out=s8[:],
                                      in0=ax[:], in1=ay[:], s0=0.0, s1=0.0,
                                      imm2=0.5)
                (nc.sync if j0 == 0 else nc.gpsimd).dma_start(
                    out=O8[ch - 2, g0:g1, :], in_=s8[j0:j1])

            # hoist all q/spf chains: DVE is in-order, so they must not sit
            # behind pair-ops that wait on ACT
            spfs = []
            for t in range(NT):
                q = work.tile([P, WP], i32, tag="q")
                nc.vector.tensor_scalar(out=q[:], in0=lps[t][:], scalar1=rb[:],
                                        scalar2=None, op0=mybir.AluOpType.mult)
                spf = keep.tile([P, WP], bf16, tag=f"spf{t}")
                nc.vector._custom_dve(FLOOR_FIX_ANT, out=spf[:],
                                      in0=q[:], in1=lps[t][:], s0=rb[:], s1=0.0,
                                      imm2=0.0)
                spfs.append(spf)

            for t in range(NT):
                j0, j1, g0, g1 = STORE[t]
                lp = lps[t]
                spf = spfs[t]
                edge_pair([(SB1r, 0), (SB1r, 2), (SB2r, 1)],
                          [(BNr, 0), (BPr, 2)], lp, 4, t, rb[:],
                          pre=(pre_psx, pre_psy) if t == 0 else None)
                edge_pair([(DPh, 0), (DPh, 1), (DPh, 2)],
                          [(BXNh, 0), (BXPh, 2)], spf, 3, t, 1.0)
                edge_pair([(SHNh, 0), (Ih, 1)],
                          [(Ih, 0), (SHNh, 1)], spf, 2, t, 1.0,
                          scale_eng=nc.gpsimd)
                al = u8p.tile([P, W], u8, tag="ax")
                for h in (0, 1):
                    pl = psum.tile([P, W // 2], f32, tag="ps")
                    _emit_filter(nc, pl[:], [(L2h, 0), (L2h, 2), (M8h, 1)],
                                 spf, h)
                    nc.scalar.activation(al[:, 1024 * h:1024 * (h + 1)], pl[:],
                                         mybir.ActivationFunctionType.Abs)
                (nc.sync if j0 == 0 else nc.gpsimd).dma_start(
                    out=O8[3, g0:g1, :], in_=al[j0:j1])
    return nc


# ------------------------------ PJRT runner ------------------------------- #

_CACHE = {}


def _get_fn():
    if "fn" in _CACHE:
        return _CACHE["fn"]
    import jax
    from jax.sharding import Mesh, PartitionSpec
    from jax.experimental.shard_map import shard_map
    from concourse import bass2jax
    from concourse.bass2jax import _bass_exec_p, partition_id_tensor

    nc = _build()
    nc.compile()
    bass2jax.install_neuronx_cc_hook()
    partition_name = nc.partition_id_tensor.name if nc.partition_id_tensor else None
    in_names, out_names, out_avals, zero_outs = [], [], [], []
    for alloc in nc.m.functions[0].allocations:
        if not isinstance(alloc, mybir.MemoryLocationSet):
            continue
        name = alloc.memorylocations[0].name
        if alloc.kind == "ExternalInput":
            if name != partition_name:
                in_names.append(name)
        elif alloc.kind == "ExternalOutput":
            shape = tuple(alloc.tensor_shape)
            dtype = mybir.dt.np(alloc.dtype)
            out_names.append(name)
            out_avals.append(jax.core.ShapedArray(shape, dtype))
            zero_outs.append(np.zeros(shape, dtype))
    n_params = len(in_names)
    all_in_names = list(in_names) + list(out_names)
    if partition_name is not None:
        all_in_names.append(partition_name)

    def _body(*args):
        operands = list(args)
        if partition_name is not None:
            operands.append(partition_id_tensor())
        outs = _bass_exec_p.bind(
            *operands,
            out_avals=tuple(out_avals),
            in_names=tuple(all_in_names),
            out_names=tuple(out_names),
            lowering_input_output_aliases=(),
            sim_require_finite=False,
            sim_require_nnan=False,
            nc=nc,
        )
        return tuple(outs)

    devices = jax.devices()[:NCORES]
    mesh = Mesh(np.asarray(devices), ("core",))
    in_specs = (PartitionSpec("core"),) * (n_params + len(out_names))
    out_specs = (PartitionSpec("core"),) * len(out_names)
    fn = jax.jit(
        shard_map(_body, mesh=mesh, in_specs=in_specs, out_specs=out_specs,
                  check_rep=False),
        keep_unused=True,
        donate_argnums=tuple(range(n_params, n_params + len(out_names))))
    info = dict(fn=fn, in_names=in_names, out_names=out_names,
                out_avals=out_avals, zero_outs=zero_outs, nc=nc)
    _CACHE["fn"] = info
    return info


def _host_inputs(pan):
    in_maps = []
    for b in range(2):
        pad = np.pad(pan[b, 0], 2, mode="reflect")  # [2052, 2052]
        for k in range(4):
            r0 = k * 512
            Xc = np.ascontiguousarray(pad[r0:r0 + ROWS, 1:1 + WP])
            bs = np.zeros((1, 2), np.float32)
            bs[0, b] = 1.0
            in_maps.append({"x": Xc, "bsel": bs})
    return in_maps


def kernel(pan: np.ndarray) -> np.ndarray:
    pan = np.asarray(pan, dtype=np.float32)
    assert pan.shape == (2, 1, 2048, 2048), pan.shape
    info = _get_fn()
    in_maps = _host_inputs(pan)
    arrs = []
    for name in info["in_names"]:
        arrs.append(np.concatenate([in_maps[c][name] for c in range(NCORES)],
                                   axis=0))
    zeros = [np.zeros((NCORES * z.shape[0], *z.shape[1:]), z.dtype)
             for z in info["zero_outs"]]
    outs = info["fn"](*arrs, *zeros)
    byname = {nm: np.asarray(a) for nm, a in zip(info["out_names"], outs)}
    oarr = byname["o"].reshape(NCORES, 2, 512, W)
    o8arr = byname["o8"].reshape(NCORES, 4, 512, W)
    norms = byname["onorms"].reshape(NCORES, 1, 2)[0, 0]
    scales = (norms / np.float32(255.0)).astype(np.float32)
    out = np.empty((2, 6, 2048, 2048), np.float32)
    for c in range(NCORES):
        b, k = c // 4, c % 4
        sl = slice(k * 512, (k + 1) * 512)
        out[b, 0:2, sl, :] = oarr[c]
        out[b, 2:6, sl, :] = o8arr[c].astype(np.float32) * scales[b]
    out[:, 0, 0, :] = 0.0   # diff_y top row (replicate pad -> 0)
    out[:, 1, :, 0] = 0.0   # diff_x left col
    return out



# revision 7
# speedup vs baseline: 1.4727x; 1.4727x over previous
"""Trainium2 Bass kernel for nn_Block1 (dense_cnn edge-filter bank).

kernel(pan) -> [2, 6, 2048, 2048] f32: concat([diff_y, diff_x, roberts,
prewitt, sobel, laplacian], axis=1) with a global-max normalization of the
Gaussian-filtered image (see the reference nn.Module).

Distribution: pure data parallel over 8 NeuronCores - core c owns rows
[512*(c%4), 512*(c%4+1)) of batch item c//4, fed a reflect-padded
[516, 2050] f32 slab. diff_y/diff_x are trivial shifted subtractions and
are produced on the host; the device computes the four conv channels as
u8 edge maps plus the per-batch norm (AllGather of per-core partial
maxes, overlapped under the norm-independent sobel matmul segment).

Math (validated vs the exact reference, rel ~2e-3):
  lp    = gauss(x) via 3 banded f32r MMs; running max via DVE reduces.
  norm  = AllGather(max) across 8 cores; rb = 255/norm.
  spf   = floor(lp*rb) via rint(lp*rb - 0.49995) -> i32 -> bf16.
  pair  = round(0.5*(clip(|cx|,0,255) + clip(|cy|,0,255))) fused in ONE
          custom DVE op per pair (inner clip exact, inner round skipped),
          u8 saturating-RNE writeback does the outer round+clip.
  sobel   on lp  (f32r banded MMs, |.| via ACT Abs to SBUF pre-norm,
                  then custom with runtime clip=norm, scale=rb/2).
  prewitt on spf (bf16 banded MMs, custom clip=255 scale=0.5).
  roberts on spf (NO matmuls: partition-shift DMA + 2 Pool TTs + custom).
  laplace on spf (bf16 banded MMs, ACT Abs -> u8).
"""
import sys

sys.path.insert(0, "/opt/trn_rl_repo")

import numpy as np
import ml_dtypes

import concourse.bacc as bacc
import concourse.mybir as mybir
import concourse.bass_isa as bass_isa
from concourse.tile import TileContext
from concourse.dve_ops import (DveOp, DveOpSpec, OPS, CUSTOM_DVE_SPECS,
                               _SUB_OPCODE_FOR_NAME, _CUSTOM_DVE_ROW_BASE)
from concourse.dve_spec import (Spec, Src0, Src1, C0, C1, Zero, lower,
                                maxx, minn)

f32 = mybir.dt.float32
f32r = mybir.dt.float32r
bf16 = mybir.dt.bfloat16
i32 = mybir.dt.int32
u8 = mybir.dt.uint8

P = 128
W = 2048
WP = 2050
ROWS = 516
NT = 5
TSTART = [0, 124, 248, 372, 388]
STORE = [(0, 124, 0, 124), (0, 124, 124, 248), (0, 124, 248, 372),
         (0, 124, 372, 496), (108, 124, 496, 512)]
NCORES = 8
H = W // 2  # 1024 psum half width


# --------------- custom DVE op (registered once per process) --------------- #

def _register(name, spec):
    if name in _SUB_OPCODE_FOR_NAME:
        for op in OPS:
            if op.name == name:
                return op
    shas = {}
    for ver in ("v3", "v4"):
        s = DveOpSpec(name=name, opcode=0, uops=lower(spec, ver=ver), rd1_en=False)
        shas[ver] = s.sha(ver)
    op = DveOp(name, spec, subdim=False, uops_sha=shas)
    OPS.append(op)
    CUSTOM_DVE_SPECS[name] = spec
    _SUB_OPCODE_FOR_NAME[name] = _CUSTOM_DVE_ROW_BASE + len(OPS) - 1
    return op


# (min(|in0|, C0) + min(|in1|, C0)) * C1 -> u8 writeback rounds+clips
PAIR_CLIP = _register("PAIR_CLIP_ANT", Spec(
    body=(minn(maxx(Src0, Zero - Src0), C0)
          + minn(maxx(Src1, Zero - Src1), C0)) * C1,
    reference=lambda in0, in1, s0, s1, imm2: (np.minimum(np.abs(in0), s0)
                                              + np.minimum(np.abs(in1), s0)) * s1,
))


# ------------------------------- bass program ------------------------------ #

def _band(c):
    """A[k, m] = c[k-m]: out[m] = sum_t c[t] x[m+t] (vertical 3-tap)."""
    A = np.zeros((P, P), np.float32)
    for m in range(P):
        for t in range(3):
            if m + t < P:
                A[m + t, m] = c[t]
    return A


def _emit_filter(nc, ps, spec, rhs, h):
    """Accumulating banded MMs for chunks [2h, 2h+1] into a [P, 1024] psum."""
    writes = [0, 0]
    total = len(spec)
    for bd, dx in spec:
        for i, c in enumerate((2 * h, 2 * h + 1)):
            first = writes[i] == 0
            writes[i] += 1
            last = writes[i] == total
            nc.tensor.matmul(ps[:, 512 * i:512 * (i + 1)], bd[:],
                             rhs[:, 512 * c + dx:512 * c + dx + 512],
                             start=first, stop=last)


def _build():
    nc = bacc.Bacc("TRN2", num_devices=NCORES)
    X = nc.dram_tensor("x", [ROWS, WP], f32, kind="ExternalInput")
    BSEL = nc.dram_tensor("bsel", [1, 2], f32, kind="ExternalInput")
    BSEL16 = nc.dram_tensor("bsel16", [1, 16], f32, kind="ExternalInput")
    O8 = nc.dram_tensor("o8", [4, 512, W], u8, kind="ExternalOutput")
    NORMS = nc.dram_tensor("onorms", [1, 1], f32, kind="ExternalOutput")
    NPP = nc.dram_tensor("onpp", [1, 16], f32, kind="ExternalOutput")

    G1m = nc.inline_tensor(_band([1, 2, 1]) / 16.0, name="G1m")
    G2m = nc.inline_tensor(_band([2, 4, 2]) / 16.0, name="G2m")
    SB1m = nc.inline_tensor(_band([-1, 0, 1]), name="SB1m")
    SB2m = nc.inline_tensor(_band([-2, 0, 2]), name="SB2m")
    BPm = nc.inline_tensor(_band([1, 2, 1]), name="BPm")
    BNm = nc.inline_tensor(_band([-1, -2, -1]), name="BNm")
    bfb = lambda c, nm: nc.inline_tensor(_band(c).astype(ml_dtypes.bfloat16),
                                         name=nm)
    DPb = bfb([1, 0, -1], "DPb")
    BXPb = bfb([1, 1, 1], "BXPb")
    BXNb = bfb([-1, -1, -1], "BXNb")
    L2b = bfb([2, 0, 2], "L2b")
    M8b = bfb([0, -8, 0], "M8b")

    with TileContext(nc) as tc:
        with tc.tile_pool(name="keep", bufs=1) as keep, \
             tc.tile_pool(name="xin", bufs=2) as xin, \
             tc.tile_pool(name="xr", bufs=2) as xrp, \
             tc.tile_pool(name="lp", bufs=5) as lpp, \
             tc.tile_pool(name="ax", bufs=4) as axp, \
             tc.tile_pool(name="qq", bufs=1) as qp, \
             tc.tile_pool(name="spf", bufs=5) as spfp, \
             tc.tile_pool(name="u", bufs=1) as up, \
             tc.tile_pool(name="rxy", bufs=1) as rxyp, \
             tc.tile_pool(name="o8", bufs=2) as o8p, \
             tc.tile_pool(name="psum", bufs=4, space="PSUM") as psum, \
             tc.tile_pool(name="dram", bufs=1, space="DRAM") as dram:

            def load_const(t, dt_):
                tl = keep.tile([P, P], dt_, tag=t.name)
                nc.sync.dma_start(out=tl[:], in_=t.ap()[:, :])
                return tl

            # gauss consts first (phase-A critical), then x loads
            masters = {}
            for m in (G1m, G2m):
                masters[m.name] = load_const(m, f32)
            xts = []
            for t in range(NT):
                xt = xin.tile([P, WP], f32, tag="x")
                eng = nc.sync if t % 2 == 0 else nc.scalar
                eng.dma_start(out=xt[:], in_=X[TSTART[t]:TSTART[t] + P, :])
                xts.append(xt)
            rbands = {}
            for nm in ("G1m", "G2m"):
                rt = keep.tile([P, P], f32r, tag=nm + "r")
                nc.vector.tensor_copy(out=rt[:], in_=masters[nm][:])
                rbands[nm] = rt

            for m in (SB1m, SB2m, BPm, BNm):
                masters[m.name] = load_const(m, f32)
            DPh = load_const(DPb, bf16)
            BXPh = load_const(BXPb, bf16)
            BXNh = load_const(BXNb, bf16)
            L2h = load_const(L2b, bf16)
            M8h = load_const(M8b, bf16)
            bsel = keep.tile([1, 2], f32, tag="bsel")
            nc.sync.dma_start(out=bsel[:], in_=BSEL[:, :])
            bselb = keep.tile([P, 2], f32, tag="bselb")
            nc.gpsimd.partition_broadcast(bselb[:], bsel[:], P)
            bs16 = keep.tile([1, 16], f32, tag="bs16")
            nc.sync.dma_start(out=bs16[:], in_=BSEL16[:, :])
            bs16b = keep.tile([P, 16], f32, tag="bs16b")
            nc.gpsimd.partition_broadcast(bs16b[:], bs16[:], P)

            for nm in ("SB1m", "SB2m", "BPm", "BNm"):
                rt = keep.tile([P, P], f32r, tag=nm + "r")
                nc.vector.tensor_copy(out=rt[:], in_=masters[nm][:])
                rbands[nm] = rt
            G1r, G2r = rbands["G1m"], rbands["G2m"]
            SB1r, SB2r = rbands["SB1m"], rbands["SB2m"]
            BPr, BNr = rbands["BPm"], rbands["BNm"]

            # ============ phase A: gauss + running max ============ #
            lps = []
            xrs = []
            macc = keep.tile([P, 1], f32, tag="macc")
            for t in range(NT):
                xr = xrp.tile([P, WP], f32r, tag="xr")
                nc.vector.tensor_copy(out=xr[:], in_=xts[t][:])
                xrs.append(xr)
                lp = lpp.tile([P, WP], f32r, tag="lp")
                for h in (0, 1):
                    ps = psum.tile([P, H], f32, tag="ps")
                    _emit_filter(nc, ps[:], [(G1r, 0), (G1r, 2), (G2r, 1)],
                                 xr, h)
                    nc.scalar.activation(lp[:, 1 + H * h:1 + H * (h + 1)],
                                         ps[:],
                                         mybir.ActivationFunctionType.Copy)
                    mt = qp.tile([P, 1], f32, tag="mt")
                    nc.vector.tensor_reduce(out=mt[:], in_=ps[:],
                                            axis=mybir.AxisListType.X,
                                            op=mybir.AluOpType.max)
                    if t == 0 and h == 0:
                        nc.vector.tensor_copy(out=macc[:], in_=mt[:])
                    else:
                        nc.vector.tensor_tensor(out=macc[:], in0=macc[:],
                                                in1=mt[:],
                                                op=mybir.AluOpType.max)
                nc.vector.tensor_copy(out=lp[:, 0:1], in_=lp[:, 2:3])
                nc.vector.tensor_copy(out=lp[:, W + 1:W + 2],
                                      in_=lp[:, W - 1:W])
                lps.append(lp)

            # ---- norm: partition reduce + AllGather (rides Pool queue,
            #      overlapped under the sobel MM segment below) ---- #
            pm = keep.tile([P, 1], f32, tag="pm")
            nc.gpsimd.partition_all_reduce(pm[:], macc[:], P,
                                           bass_isa.ReduceOp.max)
            m2 = keep.tile([1, 2], f32, tag="m2")
            nc.vector.tensor_scalar(out=m2[:], in0=bselb[0:1, :],
                                    scalar1=pm[0:1, 0:1], scalar2=None,
                                    op0=mybir.AluOpType.mult)
            ib = dram.tile([1, 2], f32)
            ob = dram.tile([1, 16], f32)
            nc.gpsimd.dma_start(ib[:], m2[:])
            nc.gpsimd.collective_compute(
                "AllGather", mybir.AluOpType.bypass,
                replica_groups=[list(range(NCORES))],
                ins=[ib.opt()], outs=[ob.opt()])
            npp = keep.tile([1, 16], f32, tag="npp")
            nc.gpsimd.dma_start(npp[:], ob[:])
            nc.gpsimd.dma_start(NPP[:, :], ob[:])
            nppb = keep.tile([P, 16], f32, tag="nppb")
            nc.gpsimd.partition_broadcast(nppb[:], npp[:], P)
            nbv = keep.tile([P, 16], f32, tag="nbv")
            nc.vector.tensor_tensor(out=nbv[:], in0=nppb[:], in1=bs16b[:],
                                    op=mybir.AluOpType.mult)
            nb = keep.tile([P, 1], f32, tag="nb")
            nc.vector.tensor_reduce(out=nb[:], in_=nbv[:],
                                    axis=mybir.AxisListType.X,
                                    op=mybir.AluOpType.max)
            rcp = keep.tile([P, 1], f32, tag="rcp")
            nc.vector.reciprocal(out=rcp[:], in_=nb[:])
            rb = keep.tile([P, 1], f32, tag="rb")
            nc.vector.tensor_scalar(out=rb[:], in0=rcp[:], scalar1=255.0,
                                    scalar2=None, op0=mybir.AluOpType.mult)
            rb2 = keep.tile([P, 1], f32, tag="rb2")
            nc.vector.tensor_scalar(out=rb2[:], in0=rcp[:], scalar1=127.5,
                                    scalar2=None, op0=mybir.AluOpType.mult)
            nc.sync.dma_start(out=NORMS[:, :], in_=nb[0:1, 0:1])

            # ==== sobel MM segment (norm-independent; overlaps collective) ==
            def sobel_mms(t):
                axs = axp.tile([P, W], f32, tag="axs")
                ays = axp.tile([P, W], f32, tag="ays")
                for h in (0, 1):
                    px = psum.tile([P, H], f32, tag="ps")
                    _emit_filter(nc, px[:], [(SB1r, 0), (SB1r, 2), (SB2r, 1)],
                                 lps[t], h)
                    nc.scalar.activation(axs[:, H * h:H * (h + 1)], px[:],
                                         mybir.ActivationFunctionType.Abs)
                for h in (0, 1):
                    py = psum.tile([P, H], f32, tag="ps")
                    _emit_filter(nc, py[:], [(BNr, 0), (BPr, 2)], lps[t], h)
                    nc.scalar.activation(ays[:, H * h:H * (h + 1)], py[:],
                                         mybir.ActivationFunctionType.Abs)
                return axs, ays

            sob = [sobel_mms(t) for t in range(4)]

            # ==== post-norm ==== #
            # spf prologue: q = rint(lp*rb - 0.49995) -> i32 -> bf16
            spfs = []
            for t in range(NT):
                q = qp.tile([P, WP], i32, tag="q")
                nc.vector.tensor_scalar(out=q[:], in0=lps[t][:],
                                        scalar1=rb[:, 0:1], scalar2=-0.49995,
                                        op0=mybir.AluOpType.mult,
                                        op1=mybir.AluOpType.add)
                spf = spfp.tile([P, WP], bf16, tag="spf")
                nc.scalar.activation(spf[:], q[:],
                                     mybir.ActivationFunctionType.Copy)
                spfs.append(spf)
                if t == 2:
                    sob.append(sobel_mms(4))  # sobel t4 rides here

            for t in range(NT):
                j0, j1, g0, g1 = STORE[t]
                spf = spfs[t]
                # sobel: fused pair custom, runtime clip=norm scale=rb/2
                axs, ays = sob[t]
                s8 = o8p.tile([P, W], u8, tag="s8")
                nc.vector._custom_dve(PAIR_CLIP, out=s8[:], in0=axs[:],
                                      in1=ays[:], s0=nb[:, 0:1],
                                      s1=rb2[:, 0:1], imm2=0.0)
                nc.sync.dma_start(out=O8[2, g0:g1, :], in_=s8[j0:j1])

                # prewitt: bf16 MMs + fused customs per half. The custom may
                # read only ONE psum operand -> stage py in SBUF via Pool.
                pw8 = o8p.tile([P, W], u8, tag="pw8")
                for h in (0, 1):
                    px = psum.tile([P, H], f32, tag="ps")
                    _emit_filter(nc, px[:], [(DPh, 0), (DPh, 1), (DPh, 2)],
                                 spf, h)
                    py = psum.tile([P, H], f32, tag="ps")
                    _emit_filter(nc, py[:], [(BXNh, 0), (BXPh, 2)], spf, h)
                    pys = rxyp.tile([P, H], f32, tag="pys")
                    nc.scalar.activation(pys[:], py[:],
                                         mybir.ActivationFunctionType.Abs)
                    nc.vector._custom_dve(PAIR_CLIP,
                                          out=pw8[:, H * h:H * (h + 1)],
                                          in0=px[:], in1=pys[:],
                                          s0=255.0, s1=0.5, imm2=0.0)
                nc.gpsimd.dma_start(out=O8[1, g0:g1, :], in_=pw8[j0:j1])

                # roberts: partition-shift DMA + Pool TTs + fused custom
                U = up.tile([P, WP], bf16, tag="U")
                nc.sync.dma_start(out=U[0:P - 1, :], in_=spf[1:P, :])
                nc.sync.dma_start(out=U[P - 1:P, :], in_=spf[P - 1:P, :])
                rx = rxyp.tile([P, W], bf16, tag="rx")
                ry = rxyp.tile([P, W], bf16, tag="ry")
                nc.gpsimd.tensor_tensor(out=rx[:], in0=U[:, 1:W + 1],
                                        in1=spf[:, 0:W],
                                        op=mybir.AluOpType.subtract)
                nc.gpsimd.tensor_tensor(out=ry[:], in0=U[:, 0:W],
                                        in1=spf[:, 1:W + 1],
                                        op=mybir.AluOpType.subtract)
                r8 = o8p.tile([P, W], u8, tag="r8")
                nc.vector._custom_dve(PAIR_CLIP, out=r8[:], in0=rx[:],
                                      in1=ry[:], s0=255.0, s1=0.5, imm2=0.0)
                nc.sync.dma_start(out=O8[0, g0:g1, :], in_=r8[j0:j1])

                # laplace: bf16 MMs + ACT Abs -> u8
                la8 = o8p.tile([P, W], u8, tag="la8")
                for h in (0, 1):
                    pl = psum.tile([P, H], f32, tag="ps")
                    _emit_filter(nc, pl[:], [(L2h, 0), (L2h, 2), (M8h, 1)],
                                 spf, h)
                    nc.scalar.activation(la8[:, H * h:H * (h + 1)], pl[:],
                                         mybir.ActivationFunctionType.Abs)
                nc.gpsimd.dma_start(out=O8[3, g0:g1, :], in_=la8[j0:j1])
    return nc


# ------------------------------ PJRT runner ------------------------------- #

_CACHE = {}


def _get_fn():
    if "fn" in _CACHE:
        return _CACHE["fn"]
    import jax
    from jax.sharding import Mesh, PartitionSpec
    from jax.experimental.shard_map import shard_map
    from concourse import bass2jax
    from concourse.bass2jax import _bass_exec_p, partition_id_tensor

    nc = _build()
    nc.compile()
    bass2jax.install_neuronx_cc_hook()
    partition_name = nc.partition_id_tensor.name if nc.partition_id_tensor else None
    in_names, out_names, out_avals, zero_outs = [], [], [], []
    for alloc in nc.m.functions[0].allocations:
        if not isinstance(alloc, mybir.MemoryLocationSet):
            continue
        name = alloc.memorylocations[0].name
        if alloc.kind == "ExternalInput":
            if name != partition_name:
                in_names.append(name)
        elif alloc.kind == "ExternalOutput":
            shape = tuple(alloc.tensor_shape)
            dtype = mybir.dt.np(alloc.dtype)
            out_names.append(name)
            out_avals.append(jax.core.ShapedArray(shape, dtype))
            zero_outs.append(np.zeros(shape, dtype))
    n_params = len(in_names)
    all_in_names = list(in_names) + list(out_names)
    if partition_name is not None:
        all_in_names.append(partition_name)

    def _body(*args):
        operands = list(args)
        if partition_name is not None:
            operands.append(partition_id_tensor())
        outs = _bass_exec_p.bind(
            *operands,
            out_avals=tuple(out_avals),
            in_names=tuple(all_in_names),
            out_names=tuple(out_names),
            lowering_input_output_aliases=(),
            sim_require_finite=False,
            sim_require_nnan=False,
            nc=nc,
        )
        return tuple(outs)

    devices = jax.devices()[:NCORES]
    mesh = Mesh(np.asarray(devices), ("core",))
    in_specs = (PartitionSpec("core"),) * (n_params + len(out_names))
    out_specs = (PartitionSpec("core"),) * len(out_names)
    fn = jax.jit(
        shard_map(_body, mesh=mesh, in_specs=in_specs, out_specs=out_specs,
                  check_rep=False),
        keep_unused=True,
        donate_argnums=tuple(range(n_params, n_params + len(out_names))))
    info = dict(fn=fn, in_names=in_names, out_names=out_names,
                out_avals=out_avals, zero_outs=zero_outs, nc=nc)
    _CACHE["fn"] = info
    return info


def _host_inputs(pan):
    in_maps = []
    for b in range(2):
        pad = np.pad(pan[b, 0], 2, mode="reflect")  # [2052, 2052]
        for k in range(4):
            r0 = k * 512
            Xc = np.ascontiguousarray(pad[r0:r0 + ROWS, 1:1 + WP])
            bs = np.zeros((1, 2), np.float32)
            bs[0, b] = 1.0
            bs16 = np.zeros((1, 16), np.float32)
            bs16[0, b::2] = 1.0
            in_maps.append({"x": Xc, "bsel": bs, "bsel16": bs16})
    return in_maps


def kernel(pan: np.ndarray) -> np.ndarray:
    pan = np.asarray(pan, dtype=np.float32)
    assert pan.shape == (2, 1, 2048, 2048), pan.shape
    info = _get_fn()
    in_maps = _host_inputs(pan)
    arrs = []
    for name in info["in_names"]:
        arrs.append(np.concatenate([in_maps[c][name] for c in range(NCORES)],
                                   axis=0))
    zeros = [np.zeros((NCORES * z.shape[0], *z.shape[1:]), z.dtype)
             for z in info["zero_outs"]]
    outs = info["fn"](*arrs, *zeros)
    byname = {nm: np.asarray(a) for nm, a in zip(info["out_names"], outs)}
    o8arr = byname["o8"].reshape(NCORES, 4, 512, W)
    norms = byname["onorms"].reshape(NCORES)
    scales = (norms / np.float32(255.0)).astype(np.float32)
    out = np.empty((2, 6, 2048, 2048), np.float32)
    for b in range(2):
        x = pan[b, 0]
        out[b, 0, 0, :] = 0.0
        out[b, 0, 1:, :] = x[1:] - x[:-1]
        out[b, 1, :, 0] = 0.0
        out[b, 1, :, 1:] = x[:, 1:] - x[:, :-1]
    for c in range(NCORES):
        b, k = c // 4, c % 4
        sl = slice(k * 512, (k + 1) * 512)
        out[b, 2:6, sl, :] = o8arr[c].astype(np.float32) * scales[c]
    return out
